# revision 3
# baseline (speedup 1.0000x reference)
"""AdaptiveMultiLoRALinear Trainium2 kernel (8 NeuronCores, data-parallel).

Math (reference):
    z = x @ W^T + b                                  # [B,S,D]
    m = sum_e scores_e * (x @ A_e @ B_e)             # low-rank adapter mix
      = x @ A_cat @ (scores-scaled B_cat)            # linearity
    gamma = min(0.5*||z|| / (||m|| + eps), 1)        # per-token clamp
    out = z + gamma * m

Distribution: pure data parallel over the B*S = 32768 tokens, 4096 tokens
per core; W / A / B replicated. No collectives.

Per-core device algorithm (tokens on PSUM partitions):
    xT [D, T] bf16 (host-transposed shard)
    z[t,o]   = sum_d xT[d,t] * Wt[d,o]      TensorE, K=D in 8 chunks
    xaT[r,t] = sum_d Ac[d,r] * xT[d,t]      TensorE (gives xa pre-transposed
                                            for the second matmul)
    m[t,o]   = sum_r xaT[r,t] * Bp[r,o]     TensorE
    ||z||^2, ||m||^2 per token via ScalarE activation(Square, accum_out)
    gamma = sqrt(min(0.25 * nz2 / (nm2 + tiny), 1))  (== min(0.5*nz/nm, 1))
    out = z + gamma*m  via ScalarE scale-copy + VectorE add
"""

import os
import numpy as np
import ml_dtypes

N_CORES = 8
BATCH, SEQ, D = 4, 8192, 1024
TOK = BATCH * SEQ              # 32768 tokens total
T = TOK // N_CORES             # 4096 tokens per core
E, RANK = 16, 16
ER = E * RANK                  # 256
P = 128
KO = D // P                    # 8 contraction chunks over D
RC = ER // P                   # 2 contraction chunks over E*r
BLK = 512                      # tokens per x block
NBLK = T // BLK                # 8
SUB = BLK // P                 # 4 token subtiles per block
NFREE = 512                    # matmul moving free-dim (one PSUM bank)
NH = D // NFREE                # 2 column groups for the 1024-wide outputs

C_CLAMP = 0.5
L_START = 0

_compiled = {}
LAST_EXEC_NS = None


def _maybe_install_ntff_hook():
    """Optional: enable NTFF profiling under axon (used when KERNEL_TRACE=1)."""
    try:
        import sys, types
        import antenv  # noqa: F401
        try:
            import antenv.axon_hooks  # noqa: F401
            return True  # already present
        except ImportError:
            pass
        from trn_agent_boot.trn_boot import _ntff_profile_via_ctypes
        hook = _ntff_profile_via_ctypes("/opt/axon/libaxon_pjrt.so")
        mod = types.ModuleType("antenv.axon_hooks")
        mod.get_axon_ntff_profile_hook = lambda: hook
        mod.set_axon_ntff_profile_hook = lambda h: None
        sys.modules["antenv.axon_hooks"] = mod
        return hook is not None
    except Exception:
        return False


def _build(use_bias: bool):
    import concourse.mybir as mybir
    import concourse.tile as tile
    from concourse import bacc

    bf = mybir.dt.bfloat16
    f32 = mybir.dt.float32
    AF = mybir.ActivationFunctionType

    nc = bacc.Bacc("TRN2", target_bir_lowering=False, debug=False,
                   num_devices=N_CORES)

    xT = nc.declare_dram_parameter("xT", [D, T], bf, isOutput=False)
    wt = nc.declare_dram_parameter("wt", [D, D], bf, isOutput=False)
    ac = nc.declare_dram_parameter("ac", [D, ER], bf, isOutput=False)
    bp = nc.declare_dram_parameter("bp", [ER, D], bf, isOutput=False)
    if use_bias:
        bvec = nc.declare_dram_parameter("bvec", [1, D], f32, isOutput=False)
    out = nc.declare_dram_parameter("out", [T, D], f32, isOutput=True)

    with tile.TileContext(nc) as tc:
        with (
            tc.tile_pool(name="weights", bufs=1) as wpool,
            tc.tile_pool(name="xin", bufs=3) as xpool,
            tc.tile_pool(name="xa", bufs=2) as xapool,
            tc.tile_pool(name="outp", bufs=4) as opool,
            tc.tile_pool(name="sq", bufs=3) as sqpool,
            tc.tile_pool(name="small", bufs=12) as spool,
            tc.tile_pool(name="pz", bufs=2, space="PSUM") as pz,
            tc.tile_pool(name="pm", bufs=2, space="PSUM") as pm,
        ):
            # Issue order matters for the pipeline head: the first xaT matmul
            # needs only ac + the first x block; the first z matmul needs wt
            # (split in column halves so the nh=0 group starts sooner).
            ac_sb = wpool.tile([P, KO, ER], bf)
            nc.sync.dma_start(out=ac_sb[:], in_=ac.rearrange("(ko p) r -> p ko r", p=P))
            wt_sb = wpool.tile([P, KO, D], bf)
            wt_r = wt.rearrange("(ko p) o -> p ko o", p=P)
            for nh in range(NH):
                ns = slice(nh * NFREE, (nh + 1) * NFREE)
                nc.sync.dma_start(out=wt_sb[:, :, ns], in_=wt_r[:, :, ns])
            bp_sb = wpool.tile([P, RC, D], bf)
            nc.sync.dma_start(out=bp_sb[:], in_=bp.rearrange("(rc p) o -> p rc o", p=P))
            if use_bias:
                b_sb = wpool.tile([P, D], f32)
                # broadcast the [1, D] bias over all 128 partitions
                import concourse.bass as bass
                b_bcast = bass.AP(tensor=bvec.ap().tensor, offset=0,
                                  ap=[[0, P], [1, D]])
                nc.sync.dma_start(out=b_sb[:], in_=b_bcast)

            xT_r = xT.rearrange("(ko p) t -> p ko t", p=P)

            for blk in range(NBLK):
                t0 = blk * BLK
                xb = xpool.tile([P, KO, BLK], bf, tag="xb")
                nc.sync.dma_start(out=xb[:], in_=xT_r[:, :, t0:t0 + BLK])

                # xaT[r, t] for the whole block (pre-transposed xa)
                xa_sb = xapool.tile([P, RC, BLK], bf, tag="xa_sb")
                for rc in range(RC):
                    # shares the pm pool slots (tag m_ps) — xa lives in PSUM
                    # only briefly before the bf16 copy to SBUF
                    xa_ps = pm.tile([P, BLK], f32, tag="m_ps")
                    for ko in range(KO):
                        nc.tensor.matmul(
                            xa_ps[:],
                            lhsT=ac_sb[:, ko, rc * P:(rc + 1) * P],
                            rhs=xb[:, ko, :],
                            start=(ko == 0), stop=(ko == KO - 1),
                        )
                    nc.vector.tensor_copy(out=xa_sb[:, rc, :], in_=xa_ps[:])

                for s in range(SUB):
                    tok = t0 + s * P
                    ts = slice(s * P, (s + 1) * P)

                    z_ps = pz.tile([P, D], f32, tag="z_ps")
                    for nh in range(NH):
                        ns = slice(nh * NFREE, (nh + 1) * NFREE)
                        for ko in range(KO):
                            nc.tensor.matmul(
                                z_ps[:, ns],
                                lhsT=xb[:, ko, ts],
                                rhs=wt_sb[:, ko, ns],
                                start=(ko == 0), stop=(ko == KO - 1),
                            )
                    m_ps = pm.tile([P, D], f32, tag="m_ps")
                    for nh in range(NH):
                        ns = slice(nh * NFREE, (nh + 1) * NFREE)
                        for rc in range(RC):
                            nc.tensor.matmul(
                                m_ps[:, ns],
                                lhsT=xa_sb[:, rc, ts],
                                rhs=bp_sb[:, rc, ns],
                                start=(rc == 0), stop=(rc == RC - 1),
                            )

                    if use_bias:
                        nc.vector.tensor_add(out=z_ps[:], in0=z_ps[:], in1=b_sb[:])

                    # per-token sum of squares on ScalarE (fused accumulate)
                    zsq = sqpool.tile([P, D], bf, tag="sq")
                    nz2 = spool.tile([P, 1], f32, tag="nz2")
                    nc.scalar.activation(out=zsq[:], in_=z_ps[:], func=AF.Square,
                                         accum_out=nz2[:])
                    msq = sqpool.tile([P, D], bf, tag="sq")
                    nm2 = spool.tile([P, 1], f32, tag="nm2")
                    nc.scalar.activation(out=msq[:], in_=m_ps[:], func=AF.Square,
                                         accum_out=nm2[:])

                    # gamma = sqrt(min(0.25*nz2/(nm2+tiny), 1))
                    rme = spool.tile([P, 1], f32, tag="rme")
                    nc.vector.tensor_scalar_add(out=rme[:], in0=nm2[:], scalar1=1e-12)
                    rm = spool.tile([P, 1], f32, tag="rm")
                    nc.vector.reciprocal(out=rm[:], in_=rme[:])
                    u = spool.tile([P, 1], f32, tag="u")
                    nc.vector.tensor_scalar(
                        out=u[:], in0=nz2[:], scalar1=rm[:],
                        scalar2=C_CLAMP * C_CLAMP,
                        op0=mybir.AluOpType.mult, op1=mybir.AluOpType.mult,
                    )
                    nc.vector.tensor_scalar_min(out=u[:], in0=u[:], scalar1=1.0)
                    gam = spool.tile([P, 1], f32, tag="gam")
                    nc.scalar.activation(out=gam[:], in_=u[:], func=AF.Sqrt)

                    # out = gamma*m + z
                    o_sb = opool.tile([P, D], f32, tag="o_sb")
                    nc.scalar.activation(out=o_sb[:], in_=m_ps[:], func=AF.Copy,
                                         scale=gam[:])
                    nc.vector.tensor_add(out=o_sb[:], in0=o_sb[:], in1=z_ps[:])
                    nc.sync.dma_start(out=out[tok:tok + P, :], in_=o_sb[:])

    nc.compile()
    return nc


def kernel(x, W, b, A, B_mat, scores, layer_idx):
    global LAST_EXEC_NS
    from concourse.bass_utils import run_bass_kernel_spmd

    x = np.asarray(x)
    W = np.asarray(W, dtype=np.float32)
    b = np.asarray(b, dtype=np.float32)
    A = np.asarray(A, dtype=np.float32)
    B_mat = np.asarray(B_mat, dtype=np.float32)
    scores = np.asarray(scores, dtype=np.float32)
    li = None if layer_idx is None else int(layer_idx)

    bf = ml_dtypes.bfloat16

    # host-side prep: transpose / concat / score-scale, cast to bf16
    tokens = np.ascontiguousarray(x.reshape(TOK, D).astype(np.float32))
    xT_full = np.ascontiguousarray(tokens.T.astype(bf))            # [D, TOK]
    wt_h = np.ascontiguousarray(W.T.astype(bf))                    # [D, D]
    ac_h = np.ascontiguousarray(A.transpose(1, 0, 2).reshape(D, ER).astype(bf))
    sc = scores if not (li is not None and li < L_START) else np.zeros_like(scores)
    bp_h = np.ascontiguousarray((sc[:, None, None] * B_mat).reshape(ER, D).astype(bf))

    use_bias = bool(np.any(b != 0.0))
    key = ("nc", use_bias)
    if key not in _compiled:
        _compiled[key] = _build(use_bias)
    nc = _compiled[key]

    in_maps = []
    for c in range(N_CORES):
        m = {
            "xT": np.ascontiguousarray(xT_full[:, c * T:(c + 1) * T]),
            "wt": wt_h,
            "ac": ac_h,
            "bp": bp_h,
        }
        if use_bias:
            m["bvec"] = np.ascontiguousarray(b.reshape(1, D))
        in_maps.append(m)

    trace = os.environ.get("KERNEL_TRACE", "0") == "1" and _maybe_install_ntff_hook()
    res = run_bass_kernel_spmd(nc, in_maps, core_ids=list(range(N_CORES)),
                               trace=bool(trace))
    LAST_EXEC_NS = res.exec_time_ns

    out = np.concatenate([res.results[c]["out"] for c in range(N_CORES)], axis=0)
    return np.ascontiguousarray(out.reshape(BATCH, SEQ, D).astype(np.float32))


# revision 5
# speedup vs baseline: 1.0258x; 1.0258x over previous
"""AdaptiveMultiLoRALinear Trainium2 kernel (8 NeuronCores, data-parallel).

Math (reference):
    z = x @ W^T + b                                  # [B,S,D]
    m = sum_e scores_e * (x @ A_e @ B_e)             # low-rank adapter mix
      = x @ A_cat @ (scores-scaled B_cat)            # linearity
    gamma = min(0.5*||z|| / (||m|| + eps), 1)        # per-token clamp
    out = z + gamma * m

Distribution: pure data parallel over the B*S = 32768 tokens, 4096 tokens
per core; W / A / B replicated. No collectives.

Per-core device algorithm (tokens on PSUM partitions):
    xT [D, T] bf16 (host-transposed shard)
    z[t,o]   = sum_d xT[d,t] * Wt[d,o]      TensorE, K=D in 8 chunks
    xaT[r,t] = sum_d Ac[d,r] * xT[d,t]      TensorE (gives xa pre-transposed
                                            for the second matmul)
    m[t,o]   = sum_r xaT[r,t] * Bp[r,o]     TensorE
    ||z||^2, ||m||^2 per token via ScalarE activation(Square, accum_out)
    gamma = sqrt(min(0.25 * nz2 / (nm2 + tiny), 1))  (== min(0.5*nz/nm, 1))
    out = z + gamma*m  via ScalarE scale-copy + VectorE add
"""

import os
import numpy as np
import ml_dtypes

N_CORES = 8
BATCH, SEQ, D = 4, 8192, 1024
TOK = BATCH * SEQ              # 32768 tokens total
T = TOK // N_CORES             # 4096 tokens per core
E, RANK = 16, 16
ER = E * RANK                  # 256
P = 128
KO = D // P                    # 8 contraction chunks over D
RC = ER // P                   # 2 contraction chunks over E*r
BLK = 512                      # tokens per x block
NBLK = T // BLK                # 8
SUB = BLK // P                 # 4 token subtiles per block
NFREE = 512                    # matmul moving free-dim (one PSUM bank)
NH = D // NFREE                # 2 column groups for the 1024-wide outputs

C_CLAMP = 0.5
L_START = 0

_compiled = {}
LAST_EXEC_NS = None


def _maybe_install_ntff_hook():
    """Optional: enable NTFF profiling under axon (used when KERNEL_TRACE=1)."""
    try:
        import sys, types
        import antenv  # noqa: F401
        try:
            import antenv.axon_hooks  # noqa: F401
            return True  # already present
        except ImportError:
            pass
        from trn_agent_boot.trn_boot import _ntff_profile_via_ctypes
        hook = _ntff_profile_via_ctypes("/opt/axon/libaxon_pjrt.so")
        mod = types.ModuleType("antenv.axon_hooks")
        mod.get_axon_ntff_profile_hook = lambda: hook
        mod.set_axon_ntff_profile_hook = lambda h: None
        sys.modules["antenv.axon_hooks"] = mod
        return hook is not None
    except Exception:
        return False


def _build(use_bias: bool):
    import concourse.mybir as mybir
    import concourse.tile as tile
    from concourse import bacc

    bf = mybir.dt.bfloat16
    f32 = mybir.dt.float32
    AF = mybir.ActivationFunctionType

    nc = bacc.Bacc("TRN2", target_bir_lowering=False, debug=False,
                   num_devices=N_CORES)

    xT = nc.declare_dram_parameter("xT", [D, T], bf, isOutput=False)
    wt = nc.declare_dram_parameter("wt", [D, D], bf, isOutput=False)
    ac = nc.declare_dram_parameter("ac", [D, ER], bf, isOutput=False)
    bp = nc.declare_dram_parameter("bp", [ER, D], bf, isOutput=False)
    if use_bias:
        bvec = nc.declare_dram_parameter("bvec", [1, D], f32, isOutput=False)
    out = nc.declare_dram_parameter("out", [T, D], f32, isOutput=True)

    with tile.TileContext(nc) as tc:
        with (
            tc.tile_pool(name="weights", bufs=1) as wpool,
            tc.tile_pool(name="xin", bufs=3) as xpool,
            tc.tile_pool(name="xa", bufs=2) as xapool,
            tc.tile_pool(name="outp", bufs=4) as opool,
            tc.tile_pool(name="sq", bufs=3) as sqpool,
            tc.tile_pool(name="small", bufs=12) as spool,
            tc.tile_pool(name="pz", bufs=2, space="PSUM") as pz,
            tc.tile_pool(name="pm", bufs=1, space="PSUM") as pm,
            tc.tile_pool(name="pxa", bufs=2, space="PSUM") as pxa,
        ):
            # Issue order matters for the pipeline head: the first xaT matmul
            # needs only ac + the first x block; the first z matmul needs wt
            # (split in column halves so the nh=0 group starts sooner).
            ac_sb = wpool.tile([P, KO, ER], bf)
            nc.sync.dma_start(out=ac_sb[:], in_=ac.rearrange("(ko p) r -> p ko r", p=P))
            wt_sb = wpool.tile([P, KO, D], bf)
            wt_r = wt.rearrange("(ko p) o -> p ko o", p=P)
            for nh in range(NH):
                ns = slice(nh * NFREE, (nh + 1) * NFREE)
                nc.sync.dma_start(out=wt_sb[:, :, ns], in_=wt_r[:, :, ns])
            bp_sb = wpool.tile([P, RC, D], bf)
            nc.sync.dma_start(out=bp_sb[:], in_=bp.rearrange("(rc p) o -> p rc o", p=P))
            if use_bias:
                b_sb = wpool.tile([P, D], f32)
                # broadcast the [1, D] bias over all 128 partitions
                import concourse.bass as bass
                b_bcast = bass.AP(tensor=bvec.ap().tensor, offset=0,
                                  ap=[[0, P], [1, D]])
                nc.sync.dma_start(out=b_sb[:], in_=b_bcast)

            xT_r = xT.rearrange("(ko p) t -> p ko t", p=P)

            for blk in range(NBLK):
                t0 = blk * BLK
                xb = xpool.tile([P, KO, BLK], bf, tag="xb")
                nc.sync.dma_start(out=xb[:], in_=xT_r[:, :, t0:t0 + BLK])

                # xaT[r, t] for the whole block (pre-transposed xa)
                xa_sb = xapool.tile([P, RC, BLK], bf, tag="xa_sb")
                for rc in range(RC):
                    xa_ps = pxa.tile([P, BLK], f32, tag="xa_ps")
                    for ko in range(KO):
                        nc.tensor.matmul(
                            xa_ps[:],
                            lhsT=ac_sb[:, ko, rc * P:(rc + 1) * P],
                            rhs=xb[:, ko, :],
                            start=(ko == 0), stop=(ko == KO - 1),
                        )
                    nc.vector.tensor_copy(out=xa_sb[:, rc, :], in_=xa_ps[:])

                for s in range(SUB):
                    tok = t0 + s * P
                    ts = slice(s * P, (s + 1) * P)

                    z_ps = pz.tile([P, D], f32, tag="z_ps")
                    for nh in range(NH):
                        ns = slice(nh * NFREE, (nh + 1) * NFREE)
                        for ko in range(KO):
                            nc.tensor.matmul(
                                z_ps[:, ns],
                                lhsT=xb[:, ko, ts],
                                rhs=wt_sb[:, ko, ns],
                                start=(ko == 0), stop=(ko == KO - 1),
                            )
                    m_ps = pm.tile([P, D], f32, tag="m_ps")
                    for nh in range(NH):
                        ns = slice(nh * NFREE, (nh + 1) * NFREE)
                        for rc in range(RC):
                            nc.tensor.matmul(
                                m_ps[:, ns],
                                lhsT=xa_sb[:, rc, ts],
                                rhs=bp_sb[:, rc, ns],
                                start=(rc == 0), stop=(rc == RC - 1),
                            )

                    if use_bias:
                        nc.vector.tensor_add(out=z_ps[:], in0=z_ps[:], in1=b_sb[:])

                    # per-token sum of squares on ScalarE (fused accumulate)
                    zsq = sqpool.tile([P, D], bf, tag="sq")
                    nz2 = spool.tile([P, 1], f32, tag="nz2")
                    nc.scalar.activation(out=zsq[:], in_=z_ps[:], func=AF.Square,
                                         accum_out=nz2[:])
                    msq = sqpool.tile([P, D], bf, tag="sq")
                    nm2 = spool.tile([P, 1], f32, tag="nm2")
                    nc.scalar.activation(out=msq[:], in_=m_ps[:], func=AF.Square,
                                         accum_out=nm2[:])

                    # gamma = sqrt(min(0.25*nz2/(nm2+tiny), 1))
                    rme = spool.tile([P, 1], f32, tag="rme")
                    nc.vector.tensor_scalar_add(out=rme[:], in0=nm2[:], scalar1=1e-12)
                    rm = spool.tile([P, 1], f32, tag="rm")
                    nc.vector.reciprocal(out=rm[:], in_=rme[:])
                    u = spool.tile([P, 1], f32, tag="u")
                    nc.vector.tensor_scalar(
                        out=u[:], in0=nz2[:], scalar1=rm[:],
                        scalar2=C_CLAMP * C_CLAMP,
                        op0=mybir.AluOpType.mult, op1=mybir.AluOpType.mult,
                    )
                    nc.vector.tensor_scalar_min(out=u[:], in0=u[:], scalar1=1.0)
                    gam = spool.tile([P, 1], f32, tag="gam")
                    nc.scalar.activation(out=gam[:], in_=u[:], func=AF.Sqrt)

                    # out = gamma*m + z
                    o_sb = opool.tile([P, D], f32, tag="o_sb")
                    nc.scalar.activation(out=o_sb[:], in_=m_ps[:], func=AF.Copy,
                                         scale=gam[:])
                    nc.vector.tensor_add(out=o_sb[:], in0=o_sb[:], in1=z_ps[:])
                    nc.sync.dma_start(out=out[tok:tok + P, :], in_=o_sb[:])

    nc.compile()
    return nc


def kernel(x, W, b, A, B_mat, scores, layer_idx):
    global LAST_EXEC_NS
    from concourse.bass_utils import run_bass_kernel_spmd

    x = np.asarray(x)
    W = np.asarray(W, dtype=np.float32)
    b = np.asarray(b, dtype=np.float32)
    A = np.asarray(A, dtype=np.float32)
    B_mat = np.asarray(B_mat, dtype=np.float32)
    scores = np.asarray(scores, dtype=np.float32)
    li = None if layer_idx is None else int(layer_idx)

    bf = ml_dtypes.bfloat16

    # host-side prep: transpose / concat / score-scale, cast to bf16
    tokens = np.ascontiguousarray(x.reshape(TOK, D).astype(np.float32))
    xT_full = np.ascontiguousarray(tokens.T.astype(bf))            # [D, TOK]
    wt_h = np.ascontiguousarray(W.T.astype(bf))                    # [D, D]
    ac_h = np.ascontiguousarray(A.transpose(1, 0, 2).reshape(D, ER).astype(bf))
    sc = scores if not (li is not None and li < L_START) else np.zeros_like(scores)
    bp_h = np.ascontiguousarray((sc[:, None, None] * B_mat).reshape(ER, D).astype(bf))

    use_bias = bool(np.any(b != 0.0))
    key = ("nc", use_bias)
    if key not in _compiled:
        _compiled[key] = _build(use_bias)
    nc = _compiled[key]

    in_maps = []
    for c in range(N_CORES):
        m = {
            "xT": np.ascontiguousarray(xT_full[:, c * T:(c + 1) * T]),
            "wt": wt_h,
            "ac": ac_h,
            "bp": bp_h,
        }
        if use_bias:
            m["bvec"] = np.ascontiguousarray(b.reshape(1, D))
        in_maps.append(m)

    trace = os.environ.get("KERNEL_TRACE", "0") == "1" and _maybe_install_ntff_hook()
    res = run_bass_kernel_spmd(nc, in_maps, core_ids=list(range(N_CORES)),
                               trace=bool(trace))
    LAST_EXEC_NS = res.exec_time_ns

    out = np.concatenate([res.results[c]["out"] for c in range(N_CORES)], axis=0)
    return np.ascontiguousarray(out.reshape(BATCH, SEQ, D).astype(np.float32))


# revision 9
# speedup vs baseline: 1.1490x; 1.1201x over previous
"""AdaptiveMultiLoRALinear Trainium2 kernel (8 NeuronCores, data-parallel).

Math (reference):
    z = x @ W^T + b                                  # [B,S,D]
    m = sum_e scores_e * (x @ A_e @ B_e)             # low-rank adapter mix
      = x @ A_cat @ (scores-scaled B_cat)            # linearity
    gamma = min(0.5*||z|| / (||m|| + eps), 1)        # per-token clamp
    out = z + gamma * m

Distribution: pure data parallel over the B*S = 32768 tokens, 4096 tokens
per core; W / A / B replicated. No collectives.

Per-core device algorithm (tokens on PSUM partitions):
    xT [D, T] bf16 (host-transposed shard)
    z[t,o]   = sum_d xT[d,t] * Wt[d,o]      TensorE, K=D in 8 chunks
    xaT[r,t] = sum_d Ac[d,r] * xT[d,t]      TensorE (gives xa pre-transposed
                                            for the second matmul)
    m[t,o]   = sum_r xaT[r,t] * Bp[r,o]     TensorE
    ||z||^2, ||m||^2 per token via ScalarE activation(Square, accum_out)
    gamma = sqrt(min(0.25 * nz2 / (nm2 + tiny), 1))  (== min(0.5*nz/nm, 1))
    out = z + gamma*m  via ScalarE scale-copy + VectorE add
"""

import os
import numpy as np
import ml_dtypes

N_CORES = 8
BATCH, SEQ, D = 4, 8192, 1024
TOK = BATCH * SEQ              # 32768 tokens total
T = TOK // N_CORES             # 4096 tokens per core
E, RANK = 16, 16
ER = E * RANK                  # 256
P = 128
KO = D // P                    # 8 contraction chunks over D
RC = ER // P                   # 2 contraction chunks over E*r
BLK = 512                      # tokens per x block
NBLK = T // BLK                # 8
SUB = BLK // P                 # 4 token subtiles per block
NFREE = 512                    # matmul moving free-dim (one PSUM bank)
NH = D // NFREE                # 2 column groups for the 1024-wide outputs

C_CLAMP = 0.5
L_START = 0

_compiled = {}
LAST_EXEC_NS = None


def _maybe_install_ntff_hook():
    """Optional: enable NTFF profiling under axon (used when KERNEL_TRACE=1)."""
    try:
        import sys, types
        import antenv  # noqa: F401
        try:
            import antenv.axon_hooks  # noqa: F401
            return True  # already present
        except ImportError:
            pass
        from trn_agent_boot.trn_boot import _ntff_profile_via_ctypes
        hook = _ntff_profile_via_ctypes("/opt/axon/libaxon_pjrt.so")
        mod = types.ModuleType("antenv.axon_hooks")
        mod.get_axon_ntff_profile_hook = lambda: hook
        mod.set_axon_ntff_profile_hook = lambda h: None
        sys.modules["antenv.axon_hooks"] = mod
        return hook is not None
    except Exception:
        return False


def _build(use_bias: bool):
    import concourse.mybir as mybir
    import concourse.tile as tile
    from concourse import bacc

    bf = mybir.dt.bfloat16
    f32 = mybir.dt.float32
    AF = mybir.ActivationFunctionType

    nc = bacc.Bacc("TRN2", target_bir_lowering=False, debug=False,
                   num_devices=N_CORES)

    xT = nc.declare_dram_parameter("xT", [D, T], bf, isOutput=False)
    wt = nc.declare_dram_parameter("wt", [D, D], bf, isOutput=False)
    ac = nc.declare_dram_parameter("ac", [D, ER], bf, isOutput=False)
    bp = nc.declare_dram_parameter("bp", [ER, D], bf, isOutput=False)
    if use_bias:
        bvec = nc.declare_dram_parameter("bvec", [1, D], f32, isOutput=False)
    out = nc.declare_dram_parameter("out", [T, D], f32, isOutput=True)

    with tile.TileContext(nc) as tc:
        with (
            tc.tile_pool(name="weights", bufs=1) as wpool,
            tc.tile_pool(name="xin", bufs=3) as xpool,
            tc.tile_pool(name="xa", bufs=2) as xapool,
            tc.tile_pool(name="outp", bufs=4) as opool,
            tc.tile_pool(name="sq", bufs=3) as sqpool,
            tc.tile_pool(name="small", bufs=12) as spool,
            tc.tile_pool(name="pz", bufs=2, space="PSUM") as pz,
            tc.tile_pool(name="pm", bufs=1, space="PSUM") as pm,
            tc.tile_pool(name="pxa", bufs=2, space="PSUM") as pxa,
        ):
            # Single HW DMA queue: issue order == service order. The first
            # xaT matmul needs only ac + xb0, so those go first; wt next
            # (first z matmul), bp last (first m matmul).
            xT_r0 = xT.rearrange("(ko p) t -> p ko t", p=P)
            ac_sb = wpool.tile([P, KO, ER], bf)
            nc.sync.dma_start(out=ac_sb[:], in_=ac.rearrange("(ko p) r -> p ko r", p=P))
            xb0 = xpool.tile([P, KO, BLK], bf, tag="xb")
            nc.sync.dma_start(out=xb0[:], in_=xT_r0[:, :, 0:BLK])
            wt_sb = wpool.tile([P, KO, D], bf)
            nc.sync.dma_start(out=wt_sb[:], in_=wt.rearrange("(ko p) o -> p ko o", p=P))
            bp_sb = wpool.tile([P, RC, D], bf)
            nc.sync.dma_start(out=bp_sb[:], in_=bp.rearrange("(rc p) o -> p rc o", p=P))
            if use_bias:
                b_sb = wpool.tile([P, D], f32)
                # broadcast the [1, D] bias over all 128 partitions
                import concourse.bass as bass
                b_bcast = bass.AP(tensor=bvec.ap().tensor, offset=0,
                                  ap=[[0, P], [1, D]])
                nc.sync.dma_start(out=b_sb[:], in_=b_bcast)

            xT_r = xT.rearrange("(ko p) t -> p ko t", p=P)

            for blk in range(NBLK):
                t0 = blk * BLK
                if blk == 0:
                    xb = xb0
                else:
                    xb = xpool.tile([P, KO, BLK], bf, tag="xb")
                    nc.sync.dma_start(out=xb[:], in_=xT_r[:, :, t0:t0 + BLK])

                # xaT[r, t] for the whole block (pre-transposed xa)
                xa_sb = xapool.tile([P, RC, BLK], bf, tag="xa_sb")
                for rc in range(RC):
                    xa_ps = pxa.tile([P, BLK], f32, tag="xa_ps")
                    for ko in range(KO):
                        nc.tensor.matmul(
                            xa_ps[:],
                            lhsT=ac_sb[:, ko, rc * P:(rc + 1) * P],
                            rhs=xb[:, ko, :],
                            start=(ko == 0), stop=(ko == KO - 1),
                        )
                    nc.vector.tensor_copy(out=xa_sb[:, rc, :], in_=xa_ps[:])

                for s in range(SUB):
                    tok = t0 + s * P
                    ts = slice(s * P, (s + 1) * P)

                    z_ps = pz.tile([P, D], f32, tag="z_ps")
                    for nh in range(NH):
                        ns = slice(nh * NFREE, (nh + 1) * NFREE)
                        for ko in range(KO):
                            nc.tensor.matmul(
                                z_ps[:, ns],
                                lhsT=xb[:, ko, ts],
                                rhs=wt_sb[:, ko, ns],
                                start=(ko == 0), stop=(ko == KO - 1),
                            )
                    m_ps = pm.tile([P, D], f32, tag="m_ps")
                    for nh in range(NH):
                        ns = slice(nh * NFREE, (nh + 1) * NFREE)
                        for rc in range(RC):
                            nc.tensor.matmul(
                                m_ps[:, ns],
                                lhsT=xa_sb[:, rc, ts],
                                rhs=bp_sb[:, rc, ns],
                                start=(rc == 0), stop=(rc == RC - 1),
                            )

                    if use_bias:
                        nc.vector.tensor_add(out=z_ps[:], in0=z_ps[:], in1=b_sb[:])

                    # per-token sum of squares on ScalarE (fused accumulate)
                    zsq = sqpool.tile([P, D], bf, tag="sq")
                    nz2 = spool.tile([P, 1], f32, tag="nz2")
                    nc.scalar.activation(out=zsq[:], in_=z_ps[:], func=AF.Square,
                                         accum_out=nz2[:])
                    msq = sqpool.tile([P, D], bf, tag="sq")
                    nm2 = spool.tile([P, 1], f32, tag="nm2")
                    nc.scalar.activation(out=msq[:], in_=m_ps[:], func=AF.Square,
                                         accum_out=nm2[:])
                    # copy m out of PSUM now (no gamma dependency) so the next
                    # subtile's m matmul isn't gated on the full gamma chain
                    m_sb = opool.tile([P, D], f32, tag="m_sb")
                    nc.vector.tensor_copy(out=m_sb[:], in_=m_ps[:])

                    # gamma = sqrt(min(0.25*nz2/(nm2+tiny), 1))
                    rme = spool.tile([P, 1], f32, tag="rme")
                    nc.vector.tensor_scalar_add(out=rme[:], in0=nm2[:], scalar1=1e-12)
                    rm = spool.tile([P, 1], f32, tag="rm")
                    nc.vector.reciprocal(out=rm[:], in_=rme[:])
                    u = spool.tile([P, 1], f32, tag="u")
                    nc.vector.tensor_scalar(
                        out=u[:], in0=nz2[:], scalar1=rm[:],
                        scalar2=C_CLAMP * C_CLAMP,
                        op0=mybir.AluOpType.mult, op1=mybir.AluOpType.mult,
                    )
                    nc.vector.tensor_scalar_min(out=u[:], in0=u[:], scalar1=1.0)
                    gam = spool.tile([P, 1], f32, tag="gam")
                    nc.scalar.activation(out=gam[:], in_=u[:], func=AF.Sqrt)

                    # out = gamma*m + z
                    o_sb = opool.tile([P, D], f32, tag="o_sb")
                    nc.scalar.activation(out=o_sb[:], in_=m_sb[:], func=AF.Copy,
                                         scale=gam[:])
                    nc.vector.tensor_add(out=o_sb[:], in0=o_sb[:], in1=z_ps[:])
                    nc.sync.dma_start(out=out[tok:tok + P, :], in_=o_sb[:])

    nc.compile()
    return nc


def kernel(x, W, b, A, B_mat, scores, layer_idx):
    global LAST_EXEC_NS
    from concourse.bass_utils import run_bass_kernel_spmd

    x = np.asarray(x)
    W = np.asarray(W, dtype=np.float32)
    b = np.asarray(b, dtype=np.float32)
    A = np.asarray(A, dtype=np.float32)
    B_mat = np.asarray(B_mat, dtype=np.float32)
    scores = np.asarray(scores, dtype=np.float32)
    li = None if layer_idx is None else int(layer_idx)

    bf = ml_dtypes.bfloat16

    # host-side prep: transpose / concat / score-scale, cast to bf16
    tokens = np.ascontiguousarray(x.reshape(TOK, D).astype(np.float32))
    xT_full = np.ascontiguousarray(tokens.T.astype(bf))            # [D, TOK]
    wt_h = np.ascontiguousarray(W.T.astype(bf))                    # [D, D]
    ac_h = np.ascontiguousarray(A.transpose(1, 0, 2).reshape(D, ER).astype(bf))
    sc = scores if not (li is not None and li < L_START) else np.zeros_like(scores)
    bp_h = np.ascontiguousarray((sc[:, None, None] * B_mat).reshape(ER, D).astype(bf))

    use_bias = bool(np.any(b != 0.0))
    key = ("nc", use_bias)
    if key not in _compiled:
        _compiled[key] = _build(use_bias)
    nc = _compiled[key]

    in_maps = []
    for c in range(N_CORES):
        m = {
            "xT": np.ascontiguousarray(xT_full[:, c * T:(c + 1) * T]),
            "wt": wt_h,
            "ac": ac_h,
            "bp": bp_h,
        }
        if use_bias:
            m["bvec"] = np.ascontiguousarray(b.reshape(1, D))
        in_maps.append(m)

    trace = os.environ.get("KERNEL_TRACE", "0") == "1" and _maybe_install_ntff_hook()
    res = run_bass_kernel_spmd(nc, in_maps, core_ids=list(range(N_CORES)),
                               trace=bool(trace))
    LAST_EXEC_NS = res.exec_time_ns

    out = np.concatenate([res.results[c]["out"] for c in range(N_CORES)], axis=0)
    return np.ascontiguousarray(out.reshape(BATCH, SEQ, D).astype(np.float32))


# revision 11
# speedup vs baseline: 1.2564x; 1.0935x over previous
"""AdaptiveMultiLoRALinear Trainium2 kernel (8 NeuronCores, data-parallel).

Math (reference):
    z = x @ W^T + b                                  # [B,S,D]
    m = sum_e scores_e * (x @ A_e @ B_e)             # low-rank adapter mix
      = x @ A_cat @ (scores-scaled B_cat)            # linearity
    gamma = min(0.5*||z|| / (||m|| + eps), 1)        # per-token clamp
    out = z + gamma * m

Distribution: pure data parallel over the B*S = 32768 tokens, 4096 tokens
per core; W / A / B replicated. No collectives.

Per-core device algorithm (tokens on PSUM partitions):
    xT [D, T] bf16 (host-transposed shard)
    z[t,o]   = sum_d xT[d,t] * Wt[d,o]      TensorE, K=D in 8 chunks
    xaT[r,t] = sum_d Ac[d,r] * xT[d,t]      TensorE (gives xa pre-transposed
                                            for the second matmul)
    m[t,o]   = sum_r xaT[r,t] * Bp[r,o]     TensorE
    ||z||^2, ||m||^2 per token via ScalarE activation(Square, accum_out)
    gamma = sqrt(min(0.25 * nz2 / (nm2 + tiny), 1))  (== min(0.5*nz/nm, 1))
    out = z + gamma*m  via ScalarE scale-copy + VectorE add
"""

import os
import numpy as np
import ml_dtypes

N_CORES = 8
BATCH, SEQ, D = 4, 8192, 1024
TOK = BATCH * SEQ              # 32768 tokens total
T = TOK // N_CORES             # 4096 tokens per core
E, RANK = 16, 16
ER = E * RANK                  # 256
P = 128
KO = D // P                    # 8 contraction chunks over D
RC = ER // P                   # 2 contraction chunks over E*r
BLK = 512                      # tokens per x block
NBLK = T // BLK                # 8
SUB = BLK // P                 # 4 token subtiles per block
NFREE = 512                    # matmul moving free-dim (one PSUM bank)
NH = D // NFREE                # 2 column groups for the 1024-wide outputs

C_CLAMP = 0.5
L_START = 0

_compiled = {}
LAST_EXEC_NS = None


def _maybe_install_ntff_hook():
    """Optional: enable NTFF profiling under axon (used when KERNEL_TRACE=1)."""
    try:
        import sys, types
        import antenv  # noqa: F401
        try:
            import antenv.axon_hooks  # noqa: F401
            return True  # already present
        except ImportError:
            pass
        from trn_agent_boot.trn_boot import _ntff_profile_via_ctypes
        hook = _ntff_profile_via_ctypes("/opt/axon/libaxon_pjrt.so")
        mod = types.ModuleType("antenv.axon_hooks")
        mod.get_axon_ntff_profile_hook = lambda: hook
        mod.set_axon_ntff_profile_hook = lambda h: None
        sys.modules["antenv.axon_hooks"] = mod
        return hook is not None
    except Exception:
        return False


def _build(use_bias: bool):
    import concourse.mybir as mybir
    import concourse.tile as tile
    from concourse import bacc

    bf = mybir.dt.bfloat16
    f32 = mybir.dt.float32
    AF = mybir.ActivationFunctionType

    nc = bacc.Bacc("TRN2", target_bir_lowering=False, debug=False,
                   num_devices=N_CORES)

    xT = nc.declare_dram_parameter("xT", [D, T], bf, isOutput=False)
    wt = nc.declare_dram_parameter("wt", [D, D], bf, isOutput=False)
    ac = nc.declare_dram_parameter("ac", [D, ER], bf, isOutput=False)
    bp = nc.declare_dram_parameter("bp", [ER, D], bf, isOutput=False)
    if use_bias:
        bvec = nc.declare_dram_parameter("bvec", [1, D], f32, isOutput=False)
    out = nc.declare_dram_parameter("out", [T, D], f32, isOutput=True)

    with tile.TileContext(nc) as tc:
        with (
            tc.tile_pool(name="weights", bufs=1) as wpool,
            tc.tile_pool(name="xin", bufs=3) as xpool,
            tc.tile_pool(name="xa", bufs=2) as xapool,
            tc.tile_pool(name="outp", bufs=4) as opool,
            tc.tile_pool(name="sq", bufs=3) as sqpool,
            tc.tile_pool(name="small", bufs=12) as spool,
            tc.tile_pool(name="pz", bufs=2, space="PSUM") as pz,
            tc.tile_pool(name="pm", bufs=1, space="PSUM") as pm,
            tc.tile_pool(name="pxa", bufs=2, space="PSUM") as pxa,
        ):
            # Single HW DMA queue: issue order == service order. The first
            # xaT matmul needs only ac + xb0, so those go first; wt next
            # (first z matmul), bp last (first m matmul).
            xT_r0 = xT.rearrange("(ko p) t -> p ko t", p=P)
            ac_sb = wpool.tile([P, KO, ER], bf)
            nc.sync.dma_start(out=ac_sb[:], in_=ac.rearrange("(ko p) r -> p ko r", p=P))
            xb0 = xpool.tile([P, KO, BLK], bf, tag="xb")
            nc.sync.dma_start(out=xb0[:], in_=xT_r0[:, :, 0:BLK])
            wt_sb = wpool.tile([P, KO, D], bf)
            nc.sync.dma_start(out=wt_sb[:], in_=wt.rearrange("(ko p) o -> p ko o", p=P))
            bp_sb = wpool.tile([P, RC, D], bf)
            nc.sync.dma_start(out=bp_sb[:], in_=bp.rearrange("(rc p) o -> p rc o", p=P))
            if use_bias:
                b_sb = wpool.tile([P, D], f32)
                # broadcast the [1, D] bias over all 128 partitions
                import concourse.bass as bass
                b_bcast = bass.AP(tensor=bvec.ap().tensor, offset=0,
                                  ap=[[0, P], [1, D]])
                nc.sync.dma_start(out=b_sb[:], in_=b_bcast)

            xT_r = xT.rearrange("(ko p) t -> p ko t", p=P)

            for blk in range(NBLK):
                t0 = blk * BLK
                if blk == 0:
                    xb = xb0
                else:
                    xb = xpool.tile([P, KO, BLK], bf, tag="xb")
                    nc.sync.dma_start(out=xb[:], in_=xT_r[:, :, t0:t0 + BLK])

                # xaT[r, t] for the whole block (pre-transposed xa)
                xa_sb = xapool.tile([P, RC, BLK], bf, tag="xa_sb")
                for rc in range(RC):
                    xa_ps = pxa.tile([P, BLK], f32, tag="xa_ps")
                    for ko in range(KO):
                        nc.tensor.matmul(
                            xa_ps[:],
                            lhsT=ac_sb[:, ko, rc * P:(rc + 1) * P],
                            rhs=xb[:, ko, :],
                            start=(ko == 0), stop=(ko == KO - 1),
                        )
                    nc.vector.tensor_copy(out=xa_sb[:, rc, :], in_=xa_ps[:])

                for s in range(SUB):
                    tok = t0 + s * P
                    ts = slice(s * P, (s + 1) * P)

                    z_ps = pz.tile([P, D], f32, tag="z_ps")
                    for nh in range(NH):
                        ns = slice(nh * NFREE, (nh + 1) * NFREE)
                        for ko in range(KO):
                            nc.tensor.matmul(
                                z_ps[:, ns],
                                lhsT=xb[:, ko, ts],
                                rhs=wt_sb[:, ko, ns],
                                start=(ko == 0), stop=(ko == KO - 1),
                            )
                    m_ps = pm.tile([P, D], f32, tag="m_ps")
                    for nh in range(NH):
                        ns = slice(nh * NFREE, (nh + 1) * NFREE)
                        for rc in range(RC):
                            nc.tensor.matmul(
                                m_ps[:, ns],
                                lhsT=xa_sb[:, rc, ts],
                                rhs=bp_sb[:, rc, ns],
                                start=(rc == 0), stop=(rc == RC - 1),
                            )

                    if use_bias:
                        nc.vector.tensor_add(out=z_ps[:], in0=z_ps[:], in1=b_sb[:])

                    # per-token sum of squares on ScalarE (fused accumulate)
                    zsq = sqpool.tile([P, D], bf, tag="sq")
                    nz2 = spool.tile([P, 1], f32, tag="nz2")
                    nc.scalar.activation(out=zsq[:], in_=z_ps[:], func=AF.Square,
                                         accum_out=nz2[:])
                    msq = sqpool.tile([P, D], bf, tag="sq")
                    nm2 = spool.tile([P, 1], f32, tag="nm2")
                    nc.scalar.activation(out=msq[:], in_=m_ps[:], func=AF.Square,
                                         accum_out=nm2[:])

                    # gamma = sqrt(min(0.25*nz2/(nm2+tiny), 1))
                    rme = spool.tile([P, 1], f32, tag="rme")
                    nc.vector.tensor_scalar_add(out=rme[:], in0=nm2[:], scalar1=1e-12)
                    rm = spool.tile([P, 1], f32, tag="rm")
                    nc.vector.reciprocal(out=rm[:], in_=rme[:])
                    u = spool.tile([P, 1], f32, tag="u")
                    nc.vector.tensor_scalar(
                        out=u[:], in0=nz2[:], scalar1=rm[:],
                        scalar2=C_CLAMP * C_CLAMP,
                        op0=mybir.AluOpType.mult, op1=mybir.AluOpType.mult,
                    )
                    nc.vector.tensor_scalar_min(out=u[:], in0=u[:], scalar1=1.0)
                    gam = spool.tile([P, 1], f32, tag="gam")
                    nc.scalar.activation(out=gam[:], in_=u[:], func=AF.Sqrt)

                    # out = gamma*m + z
                    o_sb = opool.tile([P, D], f32, tag="o_sb")
                    nc.scalar.activation(out=o_sb[:], in_=m_ps[:], func=AF.Copy,
                                         scale=gam[:])
                    nc.vector.tensor_add(out=o_sb[:], in0=o_sb[:], in1=z_ps[:])
                    nc.sync.dma_start(out=out[tok:tok + P, :], in_=o_sb[:])

    nc.compile()
    return nc


def kernel(x, W, b, A, B_mat, scores, layer_idx):
    global LAST_EXEC_NS
    from concourse.bass_utils import run_bass_kernel_spmd

    x = np.asarray(x)
    W = np.asarray(W, dtype=np.float32)
    b = np.asarray(b, dtype=np.float32)
    A = np.asarray(A, dtype=np.float32)
    B_mat = np.asarray(B_mat, dtype=np.float32)
    scores = np.asarray(scores, dtype=np.float32)
    li = None if layer_idx is None else int(layer_idx)

    bf = ml_dtypes.bfloat16

    # host-side prep: transpose / concat / score-scale, cast to bf16
    tokens = np.ascontiguousarray(x.reshape(TOK, D).astype(np.float32))
    xT_full = np.ascontiguousarray(tokens.T.astype(bf))            # [D, TOK]
    wt_h = np.ascontiguousarray(W.T.astype(bf))                    # [D, D]
    ac_h = np.ascontiguousarray(A.transpose(1, 0, 2).reshape(D, ER).astype(bf))
    sc = scores if not (li is not None and li < L_START) else np.zeros_like(scores)
    bp_h = np.ascontiguousarray((sc[:, None, None] * B_mat).reshape(ER, D).astype(bf))

    use_bias = bool(np.any(b != 0.0))
    key = ("nc", use_bias)
    if key not in _compiled:
        _compiled[key] = _build(use_bias)
    nc = _compiled[key]

    in_maps = []
    for c in range(N_CORES):
        m = {
            "xT": np.ascontiguousarray(xT_full[:, c * T:(c + 1) * T]),
            "wt": wt_h,
            "ac": ac_h,
            "bp": bp_h,
        }
        if use_bias:
            m["bvec"] = np.ascontiguousarray(b.reshape(1, D))
        in_maps.append(m)

    trace = os.environ.get("KERNEL_TRACE", "0") == "1" and _maybe_install_ntff_hook()
    res = run_bass_kernel_spmd(nc, in_maps, core_ids=list(range(N_CORES)),
                               trace=bool(trace))
    LAST_EXEC_NS = res.exec_time_ns

    out = np.concatenate([res.results[c]["out"] for c in range(N_CORES)], axis=0)
    return np.ascontiguousarray(out.reshape(BATCH, SEQ, D).astype(np.float32))
